# revision 1
# baseline (speedup 1.0000x reference)
"""nn_Attention_77541339562539 kernel: grid-window + pooled-global attention.

Self-contained. Takes FULL unsharded inputs, returns FULL output.
Sharding: pure data parallel over batch B=16 across the 8 NeuronCores
(2 batches per core); 1x1-conv weights and norm params are replicated.
"""

import numpy as np

HEAD_DIM = 64
GRID = 4
DS = 4
EPS = 1e-6

B, N, C = 16, 4096, 256
N_CORES = 8

_RUNNER = None  # cached compiled runner


def _build_runner():
    import jax
    import jax.numpy as jnp

    def conv1x1(x, w, b):
        # x: (b,C,H,W), w: (O,C), b: (O,)
        return jnp.einsum('bchw,oc->bohw', x, w) + b[None, :, None, None]

    def body(x, w_qkv, b_qkv, w_q, b_q, w_kv, b_kv, w_proj, b_proj, gn_w, gn_b):
        # x: (b_loc, N, C) slice of the batch
        b_loc = x.shape[0]
        H = W = int(np.sqrt(N))
        hd = HEAD_DIM
        nh = C // hd
        gs = GRID
        scale = hd ** -0.5

        xi = x.transpose(0, 2, 1).reshape(b_loc, C, H, W)
        qkv = conv1x1(xi, w_qkv, b_qkv)  # (b,3C,H,W)

        # grid-window attention (gs x gs windows)
        gh, gw = H // gs, W // gs
        qkv = qkv.reshape(b_loc, 3, nh, hd, gh, gs, gw, gs)
        qkv = qkv.transpose(1, 0, 2, 4, 6, 5, 7, 3).reshape(3, -1, gs * gs, hd)
        q, k, v = qkv[0], qkv[1], qkv[2]
        attn = jax.nn.softmax(jnp.einsum('wqd,wkd->wqk', q, k) * scale, axis=-1)
        grid_x = jnp.einsum('wqk,wkd->wqd', attn, v)
        grid_x = grid_x.reshape(b_loc, nh, gh, gw, gs, gs, hd)
        grid_x = grid_x.transpose(0, 1, 6, 2, 4, 3, 5).reshape(b_loc, C, H, W)

        # channels_first LayerNorm(x + grid_x)
        z = xi + grid_x
        u = z.mean(1, keepdims=True)
        s2 = ((z - u) ** 2).mean(1, keepdims=True)
        grid_x = gn_w[None, :, None, None] * ((z - u) / jnp.sqrt(s2 + EPS)) \
            + gn_b[None, :, None, None]

        # pooled global attention
        qg = conv1x1(grid_x, w_q, b_q).reshape(b_loc, nh, hd, N).transpose(0, 1, 3, 2)
        pooled = grid_x.reshape(b_loc, C, H // DS, DS, W // DS, DS).mean(axis=(3, 5))
        kv = conv1x1(pooled, w_kv, b_kv).reshape(b_loc, 2, nh, hd, -1)
        kv = kv.transpose(1, 0, 2, 4, 3)
        k, v = kv[0], kv[1]
        attn = jax.nn.softmax(jnp.einsum('bhqd,bhkd->bhqk', qg, k) * scale, axis=-1)
        global_x = jnp.einsum('bhqk,bhkd->bhqd', attn, v)
        global_x = global_x.transpose(0, 1, 3, 2).reshape(b_loc, C, H, W) + grid_x

        out = conv1x1(global_x, w_proj, b_proj)
        return out.reshape(b_loc, C, N).transpose(0, 2, 1)

    n_dev = len(jax.devices())
    if n_dev >= N_CORES:
        # data-parallel over batch across the 8 cores; weights replicated
        pfn = jax.pmap(
            body,
            in_axes=(0,) + (None,) * 10,
            devices=jax.devices()[:N_CORES],
        )

        def run(inputs):
            xs = np.ascontiguousarray(
                np.asarray(inputs['x'], dtype=np.float32).reshape(
                    N_CORES, B // N_CORES, N, C
                )
            )
            args = [xs] + [
                np.asarray(inputs[k], dtype=np.float32)
                for k in ('w_qkv', 'b_qkv', 'w_q', 'b_q', 'w_kv', 'b_kv',
                          'w_proj', 'b_proj', 'gn_w', 'gn_b')
            ]
            out = pfn(*args)
            return np.asarray(out, dtype=np.float32).reshape(B, N, C)

        return run

    jfn = jax.jit(body)

    def run(inputs):
        args = [np.asarray(inputs['x'], dtype=np.float32)] + [
            np.asarray(inputs[k], dtype=np.float32)
            for k in ('w_qkv', 'b_qkv', 'w_q', 'b_q', 'w_kv', 'b_kv',
                      'w_proj', 'b_proj', 'gn_w', 'gn_b')
        ]
        return np.asarray(jfn(*args), dtype=np.float32)

    return run


def _kernel_numpy(inputs):
    # pure-numpy fallback (correctness safety net)
    x = np.asarray(inputs['x'], dtype=np.float32)
    w_qkv = np.asarray(inputs['w_qkv'], np.float32)
    b_qkv = np.asarray(inputs['b_qkv'], np.float32)
    w_q = np.asarray(inputs['w_q'], np.float32)
    b_q = np.asarray(inputs['b_q'], np.float32)
    w_kv = np.asarray(inputs['w_kv'], np.float32)
    b_kv = np.asarray(inputs['b_kv'], np.float32)
    w_proj = np.asarray(inputs['w_proj'], np.float32)
    b_proj = np.asarray(inputs['b_proj'], np.float32)
    gn_w = np.asarray(inputs['gn_w'], np.float32)
    gn_b = np.asarray(inputs['gn_b'], np.float32)

    H = W = int(np.sqrt(N))
    hd, nh, gs = HEAD_DIM, C // HEAD_DIM, GRID
    scale = hd ** -0.5

    def conv(xc, w, b):  # (b,C,n) x (O,C) -> (b,O,n)
        return np.einsum('oc,bcn->bon', w, xc) + b[None, :, None]

    def softmax(s):
        e = np.exp(s - s.max(-1, keepdims=True))
        return e / e.sum(-1, keepdims=True)

    xi = x.transpose(0, 2, 1).reshape(B, C, N)
    qkv = conv(xi, w_qkv, b_qkv)
    gh = gw = H // gs
    q3 = qkv.reshape(B, 3, nh, hd, gh, gs, gw, gs)
    q3 = q3.transpose(1, 0, 2, 4, 6, 5, 7, 3).reshape(3, -1, gs * gs, hd)
    q, k, v = q3[0], q3[1], q3[2]
    attn = softmax(np.einsum('wqd,wkd->wqk', q, k) * scale)
    gx = np.einsum('wqk,wkd->wqd', attn, v)
    gx = gx.reshape(B, nh, gh, gw, gs, gs, hd)
    gx = gx.transpose(0, 1, 6, 2, 4, 3, 5).reshape(B, C, N)
    z = xi + gx
    u = z.mean(1, keepdims=True)
    s2 = ((z - u) ** 2).mean(1, keepdims=True)
    gx = gn_w[None, :, None] * ((z - u) / np.sqrt(s2 + EPS)) + gn_b[None, :, None]
    qg = conv(gx, w_q, b_q).reshape(B, nh, hd, N)
    pooled = gx.reshape(B, C, H // DS, DS, W // DS, DS).mean(axis=(3, 5))
    kv = conv(pooled.reshape(B, C, -1), w_kv, b_kv).reshape(B, 2, nh, hd, -1)
    kk, vv = kv[:, 0], kv[:, 1]
    attn = softmax(np.einsum('bhdq,bhdk->bhqk', qg, kk) * scale)
    glob = np.einsum('bhqk,bhdk->bhdq', attn, vv).reshape(B, C, N) + gx
    out = conv(glob, w_proj, b_proj)
    return out.transpose(0, 2, 1).astype(np.float32)


def kernel(**inputs):
    global _RUNNER
    try:
        if _RUNNER is None:
            _RUNNER = _build_runner()
        return _RUNNER(inputs)
    except Exception:
        return _kernel_numpy(inputs)


# revision 4
# speedup vs baseline: 1.1207x; 1.1207x over previous
"""nn_Attention_77541339562539 kernel: grid-window + pooled-global attention.

Self-contained. Takes FULL unsharded inputs, returns FULL output.
Sharding: pure data parallel over batch B=16 across the 8 NeuronCores
(2 batches per core); 1x1-conv weights and norm params are replicated.
"""

import numpy as np

HEAD_DIM = 64
GRID = 4
DS = 4
EPS = 1e-6

B, N, C = 16, 4096, 256
N_CORES = 8

_RUNNER = None  # cached compiled runner


def _build_runner():
    import jax
    import jax.numpy as jnp

    f32 = jnp.float32

    def bf(t):
        # bf16 matmul operands, fp32 accumulate: PE runs bf16 at 1 cycle/row
        # vs 4 for fp32; validated error impact ~2e-3 scale-relative.
        return t.astype(jnp.bfloat16)

    def conv1x1(x, w, b):
        # x: (b,C,H,W), w: (O,C), b: (O,)
        return jnp.einsum('bchw,oc->bohw', bf(x), bf(w),
                          preferred_element_type=f32) + b[None, :, None, None]

    def body(x, w_qkv, b_qkv, w_q, b_q, w_kv, b_kv, w_proj, b_proj, gn_w, gn_b):
        # x: (b_loc, N, C) slice of the batch
        b_loc = x.shape[0]
        H = W = int(np.sqrt(N))
        hd = HEAD_DIM
        nh = C // hd
        gs = GRID
        scale = hd ** -0.5

        xi = x.transpose(0, 2, 1).reshape(b_loc, C, H, W)
        qkv = conv1x1(xi, w_qkv, b_qkv)  # (b,3C,H,W)

        # grid-window attention (gs x gs windows)
        gh, gw = H // gs, W // gs
        qkv = qkv.reshape(b_loc, 3, nh, hd, gh, gs, gw, gs)
        qkv = qkv.transpose(1, 0, 2, 4, 6, 5, 7, 3).reshape(3, -1, gs * gs, hd)
        q, k, v = qkv[0], qkv[1], qkv[2]
        attn = jax.nn.softmax(
            jnp.einsum('wqd,wkd->wqk', bf(q), bf(k),
                       preferred_element_type=f32) * scale, axis=-1)
        grid_x = jnp.einsum('wqk,wkd->wqd', bf(attn), bf(v),
                            preferred_element_type=f32)
        grid_x = grid_x.reshape(b_loc, nh, gh, gw, gs, gs, hd)
        grid_x = grid_x.transpose(0, 1, 6, 2, 4, 3, 5).reshape(b_loc, C, H, W)

        # channels_first LayerNorm(x + grid_x)
        z = xi + grid_x
        u = z.mean(1, keepdims=True)
        s2 = ((z - u) ** 2).mean(1, keepdims=True)
        grid_x = gn_w[None, :, None, None] * ((z - u) / jnp.sqrt(s2 + EPS)) \
            + gn_b[None, :, None, None]

        # pooled global attention
        qg = conv1x1(grid_x, w_q, b_q).reshape(b_loc, nh, hd, N).transpose(0, 1, 3, 2)
        pooled = grid_x.reshape(b_loc, C, H // DS, DS, W // DS, DS).mean(axis=(3, 5))
        kv = conv1x1(pooled, w_kv, b_kv).reshape(b_loc, 2, nh, hd, -1)
        kv = kv.transpose(1, 0, 2, 4, 3)
        k, v = kv[0], kv[1]
        attn = jax.nn.softmax(
            jnp.einsum('bhqd,bhkd->bhqk', bf(qg), bf(k),
                       preferred_element_type=f32) * scale, axis=-1)
        global_x = jnp.einsum('bhqk,bhkd->bhqd', bf(attn), bf(v),
                              preferred_element_type=f32)
        global_x = global_x.transpose(0, 1, 3, 2).reshape(b_loc, C, H, W) + grid_x

        out = conv1x1(global_x, w_proj, b_proj)
        return out.reshape(b_loc, C, N).transpose(0, 2, 1)

    n_dev = len(jax.devices())
    if n_dev >= N_CORES:
        # data-parallel over batch across the 8 cores; weights replicated
        pfn = jax.pmap(
            body,
            in_axes=(0,) + (None,) * 10,
            devices=jax.devices()[:N_CORES],
        )

        def run(inputs):
            xs = np.ascontiguousarray(
                np.asarray(inputs['x'], dtype=np.float32).reshape(
                    N_CORES, B // N_CORES, N, C
                )
            )
            args = [xs] + [
                np.asarray(inputs[k], dtype=np.float32)
                for k in ('w_qkv', 'b_qkv', 'w_q', 'b_q', 'w_kv', 'b_kv',
                          'w_proj', 'b_proj', 'gn_w', 'gn_b')
            ]
            out = pfn(*args)
            return np.asarray(out, dtype=np.float32).reshape(B, N, C)

        return run

    jfn = jax.jit(body)

    def run(inputs):
        args = [np.asarray(inputs['x'], dtype=np.float32)] + [
            np.asarray(inputs[k], dtype=np.float32)
            for k in ('w_qkv', 'b_qkv', 'w_q', 'b_q', 'w_kv', 'b_kv',
                      'w_proj', 'b_proj', 'gn_w', 'gn_b')
        ]
        return np.asarray(jfn(*args), dtype=np.float32)

    return run


def _kernel_numpy(inputs):
    # pure-numpy fallback (correctness safety net)
    x = np.asarray(inputs['x'], dtype=np.float32)
    w_qkv = np.asarray(inputs['w_qkv'], np.float32)
    b_qkv = np.asarray(inputs['b_qkv'], np.float32)
    w_q = np.asarray(inputs['w_q'], np.float32)
    b_q = np.asarray(inputs['b_q'], np.float32)
    w_kv = np.asarray(inputs['w_kv'], np.float32)
    b_kv = np.asarray(inputs['b_kv'], np.float32)
    w_proj = np.asarray(inputs['w_proj'], np.float32)
    b_proj = np.asarray(inputs['b_proj'], np.float32)
    gn_w = np.asarray(inputs['gn_w'], np.float32)
    gn_b = np.asarray(inputs['gn_b'], np.float32)

    H = W = int(np.sqrt(N))
    hd, nh, gs = HEAD_DIM, C // HEAD_DIM, GRID
    scale = hd ** -0.5

    def conv(xc, w, b):  # (b,C,n) x (O,C) -> (b,O,n)
        return np.einsum('oc,bcn->bon', w, xc) + b[None, :, None]

    def softmax(s):
        e = np.exp(s - s.max(-1, keepdims=True))
        return e / e.sum(-1, keepdims=True)

    xi = x.transpose(0, 2, 1).reshape(B, C, N)
    qkv = conv(xi, w_qkv, b_qkv)
    gh = gw = H // gs
    q3 = qkv.reshape(B, 3, nh, hd, gh, gs, gw, gs)
    q3 = q3.transpose(1, 0, 2, 4, 6, 5, 7, 3).reshape(3, -1, gs * gs, hd)
    q, k, v = q3[0], q3[1], q3[2]
    attn = softmax(np.einsum('wqd,wkd->wqk', q, k) * scale)
    gx = np.einsum('wqk,wkd->wqd', attn, v)
    gx = gx.reshape(B, nh, gh, gw, gs, gs, hd)
    gx = gx.transpose(0, 1, 6, 2, 4, 3, 5).reshape(B, C, N)
    z = xi + gx
    u = z.mean(1, keepdims=True)
    s2 = ((z - u) ** 2).mean(1, keepdims=True)
    gx = gn_w[None, :, None] * ((z - u) / np.sqrt(s2 + EPS)) + gn_b[None, :, None]
    qg = conv(gx, w_q, b_q).reshape(B, nh, hd, N)
    pooled = gx.reshape(B, C, H // DS, DS, W // DS, DS).mean(axis=(3, 5))
    kv = conv(pooled.reshape(B, C, -1), w_kv, b_kv).reshape(B, 2, nh, hd, -1)
    kk, vv = kv[:, 0], kv[:, 1]
    attn = softmax(np.einsum('bhdq,bhdk->bhqk', qg, kk) * scale)
    glob = np.einsum('bhqk,bhdk->bhdq', attn, vv).reshape(B, C, N) + gx
    out = conv(glob, w_proj, b_proj)
    return out.transpose(0, 2, 1).astype(np.float32)


def kernel(**inputs):
    global _RUNNER
    try:
        if _RUNNER is None:
            _RUNNER = _build_runner()
        return _RUNNER(inputs)
    except Exception:
        return _kernel_numpy(inputs)
